# revision 5
# baseline (speedup 1.0000x reference)
"""TRN2 Bass kernel for nn_MixedRepeatHeads.

Math (reference): per-head proj = x @ W_proj[h] + b_proj[h]; then
  out[h] = w[h]*proj + coef[h]*caches[h] + b[h];  hidden = concat_h(out)
  result = hidden @ W_out + b_out
with w[h] = w_mix[h, index], b[h] = b_mix[h, index],
  coef[h] = w[h]*decay[h] for the first H/2 heads, decay[h] for the rest,
  decay = clip(decay_values, 0.9, 1.0) ** (1/DECAY_CONSTANT).

Folding: since H*HID == DIM, the per-head projections concatenate into one
[DIM, DIM] matmul. The per-head scalar w folds into the weight matrix, and
w*b_proj + b folds into a per-hidden-channel constant cvec. So per batch row:
  hidden = x @ Wcat_scaled + coef_vec * caches_cat + cvec
  result = hidden @ W_out + b_out

Distribution: data-parallel over batch; each of 8 cores runs two chained
[1024 x 4096 x 4096] matmuls with the cache-FMA fused into the first
matmul's PSUM eviction. Feature dims sit on partitions (batch is the moving
dim), so inputs are transposed host-side and the K dims use the library's
(pi=k%128, po=k//128, f) interleaved layout, which lets matmul1's output
feed matmul2 directly.
"""

import os
from contextlib import ExitStack

import numpy as np
import ml_dtypes

import concourse.mybir as mybir
import concourse.tile as tile
from concourse import bacc
from concourse.bass_utils import run_bass_kernel_spmd
from concourse.kernels.tile_matmul import (
    composable_matmul_tile_kernel,
    dma_from_dram_kxm,
    dma_from_dram_kxn,
    dma_to_dram_mxn,
)

B, DIM, HID, H = 8192, 4096, 256, 16
SEQ = 2048
DECAY_CONSTANT = SEQ // 512
NCORES = 8
BS = B // NCORES  # batch rows per core
P = 128
KT = DIM // P  # 32 partition-tiles along each 4096 feature dim

# matmul element type: "bf16" | "f32r" | "f32" (decided by probe_dtypes.py)
MATMUL_DT = os.environ.get("KERNEL_MATMUL_DT", "f32r")

_cache = {}


def _np_matmul_dtype():
    return {
        "bf16": ml_dtypes.bfloat16,
        "f32r": np.float32,
        "f32": np.float32,
    }[MATMUL_DT]


def _build_module(with_bout: bool):
    dt = {
        "bf16": mybir.dt.bfloat16,
        "f32r": mybir.dt.float32r,
        "f32": mybir.dt.float32,
    }[MATMUL_DT]
    f32 = mybir.dt.float32

    nc = bacc.Bacc("TRN2", target_bir_lowering=False, debug=False)

    wcat = nc.dram_tensor("wcat", (P, KT, DIM), dt, kind="ExternalInput")
    wout = nc.dram_tensor("wout", (P, KT, DIM), dt, kind="ExternalInput")
    xT = nc.dram_tensor("xT", (P, KT, BS), dt, kind="ExternalInput")
    cach = nc.dram_tensor("cach", (P, KT, BS), f32, kind="ExternalInput")
    coef = nc.dram_tensor("coef", (P, KT), f32, kind="ExternalInput")
    cvec = nc.dram_tensor("cvec", (P, KT), f32, kind="ExternalInput")
    if with_bout:
        bout = nc.dram_tensor("bout", (P, KT), f32, kind="ExternalInput")
    hidT = nc.dram_tensor("hidT", (P, KT, BS), dt)  # DRAM scratch
    outT = nc.dram_tensor("outT", (P, KT, BS), f32, kind="ExternalOutput")

    add = mybir.AluOpType.add
    mult = mybir.AluOpType.mult

    with tile.TileContext(nc) as tc:
        with ExitStack() as ctx:
            const = ctx.enter_context(tc.tile_pool(name="const", bufs=1))
            coef_sb = const.tile([P, KT], f32, tag="coef")
            cvec_sb = const.tile([P, KT], f32, tag="cvec")
            nc.sync.dma_start(coef_sb[:], coef.ap())
            nc.sync.dma_start(cvec_sb[:], cvec.ap())
            if with_bout:
                bout_sb = const.tile([P, KT], f32, tag="bout")
                nc.sync.dma_start(bout_sb[:], bout.ap())

            # ---- matmul 1: hidT = wcat.T @ xT (+ coef*cach + cvec) ----
            with ExitStack() as c1:
                cpool = c1.enter_context(tc.tile_pool(name="cachep", bufs=6))
                kxm_pool = c1.enter_context(tc.tile_pool(name="kxm1", bufs=9))
                kxn_pool = c1.enter_context(tc.tile_pool(name="kxn1", bufs=9))

                kxm_producer, kxm_shape = dma_from_dram_kxm(kxm_pool, wcat.ap())
                kxn_producer, kxn_shape = dma_from_dram_kxn(kxn_pool, xT.ap())
                mxn_consumer = dma_to_dram_mxn(hidT.ap())

                def reducer1(nc2, psum, sbuf, md):
                    po = md.m_tile_idx * md.m_subtiles + md.m_subtile_idx
                    n0 = md.n_tile_idx * md.n_tile + md.n_subtile_idx * md.n_subtile
                    ns = psum.shape[-1]
                    ct = cpool.tile([P, 512], f32, tag="cache")
                    nc2.sync.dma_start(ct[:, :ns], cach.ap()[:, po, n0 : n0 + ns])
                    nc2.vector.tensor_scalar(
                        ct[:, :ns],
                        ct[:, :ns],
                        coef_sb[:, po : po + 1],
                        cvec_sb[:, po : po + 1],
                        mult,
                        add,
                    )
                    out_view = sbuf.squeeze(1) if sbuf.ndim == 3 else sbuf
                    nc2.vector.tensor_tensor(out_view, psum, ct[:, :ns], add)

                composable_matmul_tile_kernel(
                    tc=tc,
                    kxm_shape=kxm_shape,
                    kxn_shape=kxn_shape,
                    output_type=dt,
                    kxm_producer=kxm_producer,
                    kxn_producer=kxn_producer,
                    mxn_subtile_reducer=reducer1,
                    mxn_consumer=mxn_consumer,
                )

            # ---- matmul 2: outT = wout.T @ hidT (+ b_out) ----
            with ExitStack() as c2:
                kxm_pool2 = c2.enter_context(tc.tile_pool(name="kxm2", bufs=9))
                kxn_pool2 = c2.enter_context(tc.tile_pool(name="kxn2", bufs=9))

                kxm_producer2, kxm_shape2 = dma_from_dram_kxm(kxm_pool2, wout.ap())
                kxn_producer2, kxn_shape2 = dma_from_dram_kxn(kxn_pool2, hidT.ap())
                mxn_consumer2 = dma_to_dram_mxn(outT.ap())

                if with_bout:

                    def reducer2(nc2, psum, sbuf, md):
                        po = md.m_tile_idx * md.m_subtiles + md.m_subtile_idx
                        out_view = sbuf.squeeze(1) if sbuf.ndim == 3 else sbuf
                        nc2.vector.tensor_scalar(
                            out_view, psum, bout_sb[:, po : po + 1], None, add
                        )

                else:

                    def reducer2(nc2, psum, sbuf, md):
                        nc2.any.tensor_copy(out=sbuf, in_=psum)

                composable_matmul_tile_kernel(
                    tc=tc,
                    kxm_shape=kxm_shape2,
                    kxn_shape=kxn_shape2,
                    output_type=f32,
                    kxm_producer=kxm_producer2,
                    kxn_producer=kxn_producer2,
                    mxn_subtile_reducer=reducer2,
                    mxn_consumer=mxn_consumer2,
                )

    nc.compile()
    return nc


def _interleave_k(a):
    """[K, F] -> (128, K//128, F) with pi = k % 128 innermost."""
    k, f = a.shape
    return np.ascontiguousarray(a.reshape(k // P, P, f).transpose(1, 0, 2))


def _pm_layout(v):
    """[DIM] per-channel vector -> (128, KT) with pi = c % 128."""
    return np.ascontiguousarray(v.reshape(KT, P).T)


def _prepare(
    x,
    index,
    W_proj,
    b_proj,
    W_out,
    b_out,
    w_mix,
    b_mix,
    decay_values,
    caches,
):
    x = np.asarray(x)
    W_proj = np.asarray(W_proj)
    b_proj = np.asarray(b_proj)
    W_out = np.asarray(W_out)
    b_out = np.asarray(b_out)
    w_mix = np.asarray(w_mix)
    b_mix = np.asarray(b_mix)
    decay_values = np.asarray(decay_values)
    caches = np.asarray(caches)
    idx = int(np.asarray(index))

    # per-head scalars
    w = w_mix[:, idx].astype(np.float32)  # [H]
    bmx = b_mix[:, idx].astype(np.float32)  # [H]
    decay = np.clip(decay_values.astype(np.float32), 0.9, 1.0) ** np.float32(
        1.0 / DECAY_CONSTANT
    )
    H2 = H // 2
    coef_h = np.concatenate([w[:H2] * decay[:H2], decay[H2:]]).astype(np.float32)

    w_vec = np.repeat(w, HID)  # [DIM]
    coef_vec = np.repeat(coef_h, HID)
    cvec = (w_vec * b_proj.reshape(-1).astype(np.float32)) + np.repeat(bmx, HID)

    ndt = _np_matmul_dtype()

    # Wcat[d, h*HID+j] = W_proj[h, d, j] * w[h]
    wcat = np.ascontiguousarray(W_proj.transpose(1, 0, 2)).reshape(DIM, DIM)
    wcat = wcat * w_vec[None, :]
    wcat_d = np.ascontiguousarray(_interleave_k(wcat).astype(ndt))
    wout_d = np.ascontiguousarray(_interleave_k(W_out.astype(np.float32)).astype(ndt))
    coef_d = _pm_layout(coef_vec)
    cvec_d = _pm_layout(cvec.astype(np.float32))

    with_bout = bool(np.any(b_out != 0))
    key = (MATMUL_DT, with_bout)
    if key not in _cache:
        _cache[key] = _build_module(with_bout)
    nc = _cache[key]

    in_maps = []
    for c in range(NCORES):
        sl = slice(c * BS, (c + 1) * BS)
        xT = np.ascontiguousarray(x[sl].T)  # [DIM, BS]
        xT_d = np.ascontiguousarray(_interleave_k(xT).astype(ndt))
        # cachesT[h*HID+j, b] = caches[h, b, j]
        cachT = np.ascontiguousarray(caches[:, sl, :].transpose(0, 2, 1)).reshape(
            DIM, BS
        )
        cach_d = _interleave_k(cachT.astype(np.float32))
        m = {
            "wcat": wcat_d,
            "wout": wout_d,
            "xT": xT_d,
            "cach": cach_d,
            "coef": coef_d,
            "cvec": cvec_d,
        }
        if with_bout:
            m["bout"] = _pm_layout(b_out.astype(np.float32))
        in_maps.append(m)
    return nc, in_maps


def _gather(res):
    out = np.empty((B, DIM), dtype=np.float32)
    for c in range(NCORES):
        o = res.results[c]["outT"]  # (P, KT, BS)
        # out_shard[b, m] with m = po*128 + pi
        out[c * BS : (c + 1) * BS] = (
            o.transpose(1, 0, 2).reshape(DIM, BS).T
        )
    return out


def kernel(**inputs):
    nc, in_maps = _prepare(**inputs)
    res = run_bass_kernel_spmd(nc, in_maps, core_ids=list(range(NCORES)))
    return _gather(res)


def run_traced(inputs):
    nc, in_maps = _prepare(**inputs)
    return run_bass_kernel_spmd(
        nc, in_maps, core_ids=list(range(NCORES)), trace=True
    )


if __name__ == "__main__":
    rng = np.random.default_rng(0)
    inputs = {
        "x": rng.standard_normal((B, DIM)).astype(np.float32),
        "index": 7,
        "W_proj": (rng.standard_normal((H, DIM, HID)) * 0.02).astype(np.float32),
        "b_proj": np.zeros((H, HID), np.float32),
        "W_out": (rng.standard_normal((DIM, DIM)) * 0.02).astype(np.float32),
        "b_out": np.zeros((DIM,), np.float32),
        "w_mix": np.concatenate(
            [np.full((H // 2, SEQ), 0.4, np.float32), np.full((H // 2, SEQ), -0.3, np.float32)]
        ),
        "b_mix": np.concatenate(
            [np.full((H // 2, SEQ), 3.0, np.float32), np.full((H // 2, SEQ), 0.2, np.float32)]
        ),
        "decay_values": np.ones((H,), np.float32),
        "caches": rng.standard_normal((H, B, HID)).astype(np.float32),
    }
    out = kernel(**inputs)
    print("kernel ran, out", out.shape, out.dtype)
